# revision 17
# baseline (speedup 1.0000x reference)
"""Trainium2 Bass kernel for nn_CausalFlowModel (LSTM flow model).

Model (per batch row b, B=2048 rows total):
  h0 = MLP_enc(x[b])                         # 8 -> 256 -> 256 -> 64, tanh/tanh/linear
  h0_stack = [x[b]; h0]                      # 72
  run LSTM (input 9, hidden 72) over T=512 steps from (h0_stack, c0=0)
  dec_in = (1-d)*h[l-2] + d*h[l-1]           # l = h_lens[b], d = deltas[b, l-1]
  out[b] = MLP_dec(dec_in)                   # 72 -> 288 -> 288 -> 8, tanh/tanh/linear

Mapping: pure data parallel over 8 cores (256 rows/core). On-chip layout is
transposed: state tiles are [feature, batch_col].  Per step one fused matmul
(K = 72 h + 9 x + 1 ones = 82) produces the four gates [72, 256] each in PSUM;
sigmoid over [F|I|O] in one ACT op, tanh(G), then the cell update on DVE.

Rows are sorted by h_lens (ascending) and dealt round-robin to cores.  The
h[l-1] / h[l-2] captures are narrow windowed copies on the GpSimd engine whose
column offset is loaded at runtime from a per-core offset table, so the IR is
input-independent.
"""

import math
import os

import numpy as np

import concourse.bacc as bacc
import concourse.bass as bass
import concourse.mybir as mybir
import concourse.tile as tile
from concourse.bass_utils import run_bass_kernel_spmd

F32 = mybir.dt.float32
I32 = mybir.dt.int32
AF = mybir.ActivationFunctionType
ALU = mybir.AluOpType

# Problem constants
B, T, SD, CD = 2048, 512, 8, 8
H = 72          # control_rnn_size + state_dim
NCORES = 8
BC = B // NCORES  # 256 batch columns per core


class Cfg:
    def __init__(self, T=T, bc=BC, W=16, R=4, mm_dt=mybir.dt.float32r,
                 ncores=NCORES):
        self.T = T          # number of LSTM steps
        self.bc = bc        # batch columns per core
        self.W = W          # capture window width
        self.R = R          # rotating hx buffers
        self.mm_dt = mm_dt  # matmul dtype (float32 or float32r)
        self.ncores = ncores
        self.pad = bc + W   # padded column width of capture-read tiles


# --------------------------------------------------------------------------- #
# host-side preparation
# --------------------------------------------------------------------------- #

def _gate_reorder(w4h: np.ndarray) -> np.ndarray:
    """Reorder [4H, ...] from pytorch gate order (i,f,g,o) to (f,i,o,g)."""
    i, f, g, o = np.split(w4h, 4, axis=0)
    return np.concatenate([f, i, o, g], axis=0)


def host_prep(inputs: dict, cfg: Cfg):
    """Sort/deal rows, build per-core input maps (shared weight arrays)."""
    lens = np.asarray(inputs["h_lens"]).astype(np.int64)
    order = np.argsort(lens, kind="stable")

    # shared (replicated) weight tensors ------------------------------------
    W_ih = np.asarray(inputs["W_ih"], np.float32)   # [288, 9]
    W_hh = np.asarray(inputs["W_hh"], np.float32)   # [288, 72]
    b_g = np.asarray(inputs["b_ih"], np.float32) + np.asarray(inputs["b_hh"], np.float32)
    Wg = np.concatenate([W_hh, W_ih, b_g[:, None]], axis=1)   # [288, 82]
    Wg = _gate_reorder(Wg)                                    # (f,i,o,g)
    wg_all = np.ascontiguousarray(Wg.T)                       # [82, 288] lhsT

    def kchunks(wT, chunk=128):
        # split [K, M] along K into <=128 rows
        return [np.ascontiguousarray(wT[s:s + chunk])
                for s in range(0, wT.shape[0], chunk)]

    we1T = np.ascontiguousarray(np.asarray(inputs["enc_W1"], np.float32).T)  # [8, 256]
    we2T = np.ascontiguousarray(np.asarray(inputs["enc_W2"], np.float32).T)  # [256, 256]
    we3T = np.ascontiguousarray(np.asarray(inputs["enc_W3"], np.float32).T)  # [256, 64]
    wd1T = np.ascontiguousarray(np.asarray(inputs["dec_W1"], np.float32).T)  # [72, 288]
    wd2T = np.ascontiguousarray(np.asarray(inputs["dec_W2"], np.float32).T)  # [288, 288]
    wd3T = np.ascontiguousarray(np.asarray(inputs["dec_W3"], np.float32).T)  # [288, 8]

    def bias_cols(b, p=128):
        # [M] -> [p, ceil(M/p)] column-chunked per-partition bias
        ncol = (len(b) + p - 1) // p
        out = np.zeros((p, ncol), np.float32)
        for c in range(ncol):
            seg = b[c * p:(c + 1) * p]
            out[: len(seg), c] = seg
        return out

    shared = {
        "wg_all": wg_all,
        "we1T": we1T,
        "we2Tk0": kchunks(we2T)[0], "we2Tk1": kchunks(we2T)[1],
        "we3Tk0": kchunks(we3T)[0], "we3Tk1": kchunks(we3T)[1],
        "wd1T": wd1T,
        "wd2Tk0": kchunks(wd2T)[0], "wd2Tk1": kchunks(wd2T)[1],
        "wd2Tk2": kchunks(wd2T)[2],
        "wd3Tk0": kchunks(wd3T)[0], "wd3Tk1": kchunks(wd3T)[1],
        "wd3Tk2": kchunks(wd3T)[2],
        "be1": bias_cols(np.asarray(inputs["enc_b1"], np.float32)),
        "be2": bias_cols(np.asarray(inputs["enc_b2"], np.float32)),
        "be3": bias_cols(np.asarray(inputs["enc_b3"], np.float32), p=64),
        "bd1": bias_cols(np.asarray(inputs["dec_b1"], np.float32)),
        "bd2": bias_cols(np.asarray(inputs["dec_b2"], np.float32)),
        "bd3": bias_cols(np.asarray(inputs["dec_b3"], np.float32), p=8),
    }

    x = np.asarray(inputs["x"], np.float32)
    rnn = np.asarray(inputs["rnn_input"], np.float32)
    deltas = np.asarray(inputs["deltas"], np.float32)

    in_maps, perms = [], []
    maxw = 0
    for k in range(cfg.ncores):
        perm = order[np.arange(cfg.bc) * cfg.ncores + k]
        perms.append(perm)
        lk = lens[perm]
        # capture offset table: tb[u] = #cols with len <= u, u in [0, T+2)
        tb = np.searchsorted(lk, np.arange(cfg.T + 4), side="right").astype(np.int32)
        maxw = max(maxw, int(np.max(np.bincount(lk, minlength=1))))
        dsel = deltas[perm, lk - 1, 0].astype(np.float32)        # [bc]
        d1 = np.broadcast_to(dsel, (H, cfg.bc)).copy()           # weight for h[l-1]
        d2 = np.broadcast_to(1.0 - dsel, (H, cfg.bc)).copy()     # weight for h[l-2]
        rk = rnn[perm].transpose(1, 2, 0)                        # [T, 9, bc]
        rk = np.concatenate(
            [rk, np.ones((cfg.T, 1, cfg.bc), np.float32)], axis=1)   # + ones row
        m = dict(shared)
        m.update({
            "xT": np.ascontiguousarray(x[perm].T),               # [8, bc]
            "rnnT": np.ascontiguousarray(rk).reshape(cfg.T * (CD + 2), cfg.bc),
            "tb": tb.reshape(1, -1),
            "d1": d1,
            "d2": d2,
        })
        in_maps.append(m)
    assert maxw <= cfg.W, f"capture window too small: need {maxw} > {cfg.W}"
    return in_maps, perms


# --------------------------------------------------------------------------- #
# device kernel
# --------------------------------------------------------------------------- #

def build_nc(cfg: Cfg):
    nc = bacc.Bacc("TRN2", target_bir_lowering=False, debug=False,
                   enable_asserts=False, num_devices=cfg.ncores)
    T, bc, W, R, PAD = cfg.T, cfg.bc, cfg.W, cfg.R, cfg.pad

    RD = cfg.mm_dt  # dtype of every matmul operand

    def din(name, shape=None, dt=F32):
        return nc.dram_tensor(name, list(shape), dt, kind="ExternalInput").ap()

    ins = {
        "xT": din("xT", [SD, bc], RD),
        "rnnT": din("rnnT", [T * (CD + 2), bc], RD),
        "tb": din("tb", [1, T + 4], I32),
        "d1": din("d1", [H, bc]),
        "d2": din("d2", [H, bc]),
        "wg_all": din("wg_all", dt=RD, shape=[H + CD + 2, 4 * H]),
        "we1T": din("we1T", dt=RD, shape=[SD, 256]),
        "we2Tk0": din("we2Tk0", dt=RD, shape=[128, 256]), "we2Tk1": din("we2Tk1", dt=RD, shape=[128, 256]),
        "we3Tk0": din("we3Tk0", dt=RD, shape=[128, 64]), "we3Tk1": din("we3Tk1", dt=RD, shape=[128, 64]),
        "wd1T": din("wd1T", dt=RD, shape=[H, 288]),
        "wd2Tk0": din("wd2Tk0", dt=RD, shape=[128, 288]), "wd2Tk1": din("wd2Tk1", dt=RD, shape=[128, 288]),
        "wd2Tk2": din("wd2Tk2", dt=RD, shape=[32, 288]),
        "wd3Tk0": din("wd3Tk0", dt=RD, shape=[128, SD]), "wd3Tk1": din("wd3Tk1", dt=RD, shape=[128, SD]),
        "wd3Tk2": din("wd3Tk2", dt=RD, shape=[32, SD]),
        "be1": din("be1", [128, 2]), "be2": din("be2", [128, 2]),
        "be3": din("be3", [64, 1]),
        "bd1": din("bd1", [128, 3]), "bd2": din("bd2", [128, 3]),
        "bd3": din("bd3", [SD, 1]),
    }
    out_dram = nc.dram_tensor("out", [SD, bc], F32, kind="ExternalOutput").ap()

    KG = H + CD + 2  # 82: [h(72); x(9); ones(1)]

    with tile.TileContext(nc) as tc:
        with tc.tile_pool(name="const", bufs=1) as cpool, \
             tc.tile_pool(name="work", bufs=2) as wpool, \
             tc.tile_pool(name="dram", bufs=2, space="DRAM") as dpool, \
             tc.tile_pool(name="psum", bufs=2, space="PSUM") as ppool:

            # ---- load constants -------------------------------------------
            sb = {}
            for name in ["wg_all", "we1T", "we2Tk0", "we2Tk1", "we3Tk0",
                         "we3Tk1", "wd1T", "wd2Tk0", "wd2Tk1", "wd2Tk2",
                         "wd3Tk0", "wd3Tk1", "wd3Tk2", "be1", "be2", "be3",
                         "bd1", "bd2", "bd3", "d1", "d2", "tb"]:
                ap = ins[name]
                t_ = cpool.tile(list(ap.shape), ap.dtype, name=f"sb_{name}")
                nc.sync.dma_start(t_, ap)
                sb[name] = t_

            # persistent state tiles
            hx = [cpool.tile([KG, PAD], RD, name=f"hx{r}") for r in range(R)]
            CG = cpool.tile([H, 2 * bc], F32, name="CG")   # [c | tanh(g)]
            H1d = nc.dram_tensor("H1d", [H, PAD], F32, kind="ExternalOutput").ap()
            H2d = nc.dram_tensor("H2d", [H, PAD], F32, kind="ExternalOutput").ap()
            hdx = nc.dram_tensor("hdx", [H, PAD], F32, kind="ExternalOutput").ap()
            H1 = cpool.tile([H, PAD], F32, name="H1")      # h[l-1] capture
            H2 = cpool.tile([H, PAD], F32, name="H2")      # h[l-2] capture
            nc.vector.memset(CG[:, 0:bc], 0.0)                      # c0 = 0
            nc.vector.memset(H1, 0.0)
            nc.vector.memset(H2, 0.0)
            for r in range(R):
                nc.vector.memset(hx[r].bitcast(F32), 0.0)

            def mm(out, lhsT, rhs, start=True, stop=True):
                nc.tensor.matmul(out, lhsT, rhs, start=start, stop=stop)

            # ---- encoder MLP: h0 = W3 t(W2 t(W1 x + b1) + b2) + b3 --------
            # x lives in hx[0][0:8, 0:bc]
            nc.sync.dma_start(hx[0][0:SD, 0:bc], ins["xT"])
            ez1p = ppool.tile([128, 512], F32, name="ez1p", tag="ps")
            for c in range(2):
                mm(ez1p[:, 256 * c:256 * (c + 1)],
                   sb["we1T"][:, 128 * c:128 * (c + 1)], hx[0][0:SD, 0:bc])
            ez1 = wpool.tile([128, 512], RD, name="ez1")
            for c in range(2):
                nc.scalar.activation(ez1[:, 256 * c:256 * (c + 1)],
                                     ez1p[:, 256 * c:256 * (c + 1)],
                                     AF.Tanh, bias=sb["be1"][:, c:c + 1])
            ez2p = ppool.tile([128, 512], F32, name="ez2p", tag="ps")
            for c in range(2):
                for k in range(2):
                    mm(ez2p[:, 256 * c:256 * (c + 1)],
                       sb[f"we2Tk{k}"][:, 128 * c:128 * (c + 1)],
                       ez1[:, 256 * k:256 * (k + 1)],
                       start=(k == 0), stop=(k == 1))
            ez2 = wpool.tile([128, 512], RD, name="ez2")
            for c in range(2):
                nc.scalar.activation(ez2[:, 256 * c:256 * (c + 1)],
                                     ez2p[:, 256 * c:256 * (c + 1)],
                                     AF.Tanh, bias=sb["be2"][:, c:c + 1])
            eh0p = ppool.tile([64, 256], F32, name="eh0p", tag="ps")
            for k in range(2):
                mm(eh0p, sb[f"we3Tk{k}"], ez2[:, 256 * k:256 * (k + 1)],
                   start=(k == 0), stop=(k == 1))
            eh0 = wpool.tile([64, 256], RD, name="eh0")
            nc.scalar.activation(eh0, eh0p, AF.Identity, bias=sb["be3"][:, 0:1])
            # shift h0 into hx[0] rows 8:72 (partition shift -> DMA)
            nc.sync.dma_start(hx[0][SD:H, 0:bc], eh0)

            # ---- LSTM over T steps ----------------------------------------
            rnn_ap = ins["rnnT"]
            NX = CD + 2
            for r in range(min(R, T)):
                nc.sync.dma_start(hx[r][H:KG, 0:bc],
                                  rnn_ap[r * NX:(r + 1) * NX, :])

            dbg = os.environ.get("KDBG", "")
            prev_cap = None
            for t in range(T):
                cur = hx[t % R]
                nxt = hx[(t + 1) % R]
                rhs = cur[0:KG, 0:bc]
                gp = ppool.tile([H, 4 * bc], F32, name="gp", tag="ps")
                for c in range(4):  # F, I, O, G
                    mm(gp[:, bc * c:bc * (c + 1)],
                       sb["wg_all"][:, H * c:H * (c + 1)], rhs)
                S = wpool.tile([H, 3 * bc], F32, name="S")
                nc.scalar.activation(S, gp[:, 0:3 * bc], AF.Sigmoid)
                nc.scalar.activation(CG[:, bc:2 * bc], gp[:, 3 * bc:4 * bc],
                                     AF.Tanh)
                Tt = wpool.tile([H, 2 * bc], F32, name="Tt")
                nc.vector.tensor_tensor(Tt, S[:, 0:2 * bc], CG, op=ALU.mult)
                nc.vector.tensor_tensor(CG[:, 0:bc], Tt[:, 0:bc],
                                        Tt[:, bc:2 * bc], op=ALU.add)
                TC = wpool.tile([H, bc], F32, name="TC")
                nc.scalar.activation(TC, CG[:, 0:bc], AF.Tanh)
                nc.vector.tensor_tensor(nxt[0:H, 0:bc], S[:, 2 * bc:3 * bc],
                                        TC, op=ALU.mult)
                # prefetch x for step t+R into the buffer just read
                if t + R < T:
                    nc.sync.dma_start(
                        cur[H:KG, 0:bc],
                        rnn_ap[(t + R) * NX:(t + R + 1) * NX, :])
                # captures: h_t -> DRAM ring; h[l-1] (tb[t]) and h[l-2]
                # (tb[t+1]) as dynamic-offset DRAM->DRAM window copies
                hd = hdx
                nc.sync.dma_start(hd, nxt.bitcast(F32)[0:H, 0:PAD])
                if "nowin" in dbg:
                    prev_cap = None
                    continue
                if "statwin" in dbg:
                    nc.sync.dma_start(H1d[:, 0:W], hd[:, 0:W])
                    nc.sync.dma_start(H2d[:, 0:W], hd[:, 0:W])
                    prev_cap = None
                    continue
                tmp1 = nc.sync.alloc_register(f"cap1_{t}")
                if "movwin" in dbg:
                    ld1 = nc.sync.reg_mov(tmp1, 0)
                else:
                    ld1 = nc.sync.reg_load(tmp1, sb["tb"][0:1, t:t + 1])
                if prev_cap is not None:
                    tile.add_dep_helper(ld1.ins, prev_cap.ins, sync=False,
                                        reason="cap reg chain")
                v1 = nc.sync.snap(tmp1, donate=True)
                tmp2 = nc.sync.alloc_register(f"cap2_{t}")
                if "movwin" in dbg:
                    ld2 = nc.sync.reg_mov(tmp2, 0)
                else:
                    ld2 = nc.sync.reg_load(tmp2, sb["tb"][0:1, t + 1:t + 2])
                v2 = nc.sync.snap(tmp2, donate=True)
                cp1 = nc.sync.dma_start(H1d[:, bass.ds(v1, W)],
                                        hd[:, bass.ds(v1, W)],
                                        cond=None if "nocond" in dbg
                                        else (v2 > v1))
                prev_cap = nc.sync.dma_start(H2d[:, bass.ds(v2, W)],
                                             hd[:, bass.ds(v2, W)])

            nc.sync.dma_start(H1[:, 0:bc], H1d[:, 0:bc])
            nc.sync.dma_start(H2[:, 0:bc], H2d[:, 0:bc])

            # ---- dec_in = d1*h[l-1] + d2*h[l-2] ---------------------------
            U1 = wpool.tile([H, bc], F32, name="U1")
            nc.vector.tensor_tensor(U1, sb["d1"], H1[:, 0:bc], op=ALU.mult)
            U2 = wpool.tile([H, bc], F32, name="U2")
            nc.vector.tensor_tensor(U2, sb["d2"], H2[:, 0:bc], op=ALU.mult)
            DI = wpool.tile([H, bc], RD, name="DI")
            nc.vector.tensor_tensor(DI, U1, U2, op=ALU.add)

            # ---- decoder MLP ----------------------------------------------
            CH1 = [(0, 128), (128, 128), (256, 32)]
            dz1p = ppool.tile([128, 768], F32, name="dz1p", tag="ps")
            for c, (off, m) in enumerate(CH1):
                mm(dz1p[0:m, 256 * c:256 * c + bc], sb["wd1T"][:, off:off + m], DI)
            dz1 = wpool.tile([128, 768], RD, name="dz1")
            for c, (off, m) in enumerate(CH1):
                nc.scalar.activation(dz1[0:m, 256 * c:256 * c + bc],
                                     dz1p[0:m, 256 * c:256 * c + bc],
                                     AF.Tanh, bias=sb["bd1"][0:m, c:c + 1])
            dz2p = ppool.tile([128, 768], F32, name="dz2p", tag="ps")
            for c, (off, m) in enumerate(CH1):
                for k, (koff, km) in enumerate(CH1):
                    mm(dz2p[0:m, 256 * c:256 * c + bc],
                       sb[f"wd2Tk{k}"][0:km, off:off + m],
                       dz1[0:km, 256 * k:256 * k + bc],
                       start=(k == 0), stop=(k == 2))
            dz2 = wpool.tile([128, 768], RD, name="dz2")
            for c, (off, m) in enumerate(CH1):
                nc.scalar.activation(dz2[0:m, 256 * c:256 * c + bc],
                                     dz2p[0:m, 256 * c:256 * c + bc],
                                     AF.Tanh, bias=sb["bd2"][0:m, c:c + 1])
            dz3p = ppool.tile([SD, 256], F32, name="dz3p", tag="ps")
            for k, (koff, km) in enumerate(CH1):
                mm(dz3p, sb[f"wd3Tk{k}"][0:km, :],
                   dz2[0:km, 256 * k:256 * k + bc],
                   start=(k == 0), stop=(k == 2))
            OUT = wpool.tile([SD, bc], F32, name="OUT")
            nc.scalar.activation(OUT, dz3p, AF.Identity, bias=sb["bd3"][:, 0:1])
            nc.sync.dma_start(out_dram, OUT)

    nc.compile()
    return nc, ins, out_dram


# --------------------------------------------------------------------------- #
# entry point
# --------------------------------------------------------------------------- #

def kernel(**inputs) -> np.ndarray:
    cfg = Cfg()
    lens = np.asarray(inputs["h_lens"]).astype(np.int64)
    maxcnt = int(np.max(np.bincount(lens, minlength=1)))
    if maxcnt > cfg.W:
        cfg.W = 1 << int(math.ceil(math.log2(maxcnt)))
        cfg.pad = cfg.bc + cfg.W
    in_maps, perms = host_prep(inputs, cfg)
    nc, _, _ = build_nc(cfg)
    res = run_bass_kernel_spmd(nc, in_maps, core_ids=list(range(cfg.ncores)))
    out = np.empty((B, SD), np.float32)
    for k in range(cfg.ncores):
        out[perms[k]] = res.results[k]["out"].T
    return out
